# revision 36
# baseline (speedup 1.0000x reference)
"""MixHop layer (3 hops) on 8 Trainium2 NeuronCores.

out = concat_j [ adj_t^j @ (x @ W_j.T + b_j) ]   for j = 0,1,2

Strategy (destination sharding, one SPMD program on 8 cores), tuned to
minimize host<->device transfer over the axon tunnel (the wall-clock
bottleneck) and device DMA/Pool time (the on-chip bottleneck):
  - Each core receives ONLY its own x shard (fp16, host-transposed), the
    replicated [3,128,128] weights, and compact per-core edge encodings.
  - Phase AB: one pass over the shard computes y0 = x@W0.T+b0 (int8 out,
    per-row scales) and the projection shard [z1|z2] = x@[W1|W2].T+b (fp16).
  - AllGather (device, NeuronLink) assembles the full [N,256] fp16 table.
  - Phase C (SpMM1): dma_gather 512B fp16 table rows per in-edge
    (block-major chunk stream), build the one-hot*weight segment matrix S
    on device (tensor_scalar is_equal+mult against an fp16 iota tile, DVE
    2x mode), segment-sum via PE matmuls accumulated in PSUM.  Per block
    the PSUM result is quantized to int8 (-> q cols 128:256) and copied
    fp16 (-> z2 block buffer); both are plain block-order DMA writes --
    no scatter: outputs stay in block-slot order and the HOST undoes the
    permutation (it computed it), while pass 2's gather indices are
    encoded directly in the block-permuted z2 coordinate system.
  - Phase D: AllGather z2 block buffers -> permuted z2 table [NC*NBLK*128].
  - Phase E (SpMM2): gathers permuted z2 rows -> out2 (int8, q cols
    256:384, per-row scales).
Outputs are int8 with per-row absmax scales in block-slot layout
[128, 3*NBLK]; row-wise int8 adds ~6e-3 fro error vs the fp32 reference,
well inside the 2e-2 gate.  Gather index streams upload in compact
[16, n/16] form and are replicated to the 128-partition layout on device.
"""

import sys

sys.path.insert(0, "/opt/trn_rl_repo")

import numpy as np

import concourse.tile as tile
from concourse import bacc, mybir
from concourse import bass_utils
from concourse import bass2jax as _b2j

P = 128
QMAX = 126.5

# --- patched PJRT runner -------------------------------------------------
# The stock bass2jax.run_bass_via_pjrt rebuilds the jit closure on every
# call (full retrace + compile-cache lookup, ~0.4s) and converts each
# output array once per core (n_outs*n_cores blocking device fetches,
# ~0.8s).  Memoize the jitted runner per Bass module and fetch each
# output exactly once; semantics are identical.
_RUNNER_CACHE = {}
_CONCAT_CACHE = {}
_STREAM_HOOK = [None]   # optional per-core shard consumer (kernel() sets it)


def _patched_run_bass_via_pjrt(nc, in_maps, n_cores):
    import jax

    ent = _RUNNER_CACHE.get(id(nc))
    if ent is None:
        _b2j.install_neuronx_cc_hook()
        assert nc.dbg_addr is None, "patched runner assumes debug=False"
        pname = nc.partition_id_tensor.name if nc.partition_id_tensor else None
        in_names, out_names, out_avals = [], [], []
        for alloc in nc.m.functions[0].allocations:
            if not isinstance(alloc, mybir.MemoryLocationSet):
                continue
            name = alloc.memorylocations[0].name
            if alloc.kind == "ExternalInput":
                if name != pname:
                    in_names.append(name)
            elif alloc.kind == "ExternalOutput":
                out_names.append(name)
                out_avals.append(jax.core.ShapedArray(
                    tuple(alloc.tensor_shape), mybir.dt.np(alloc.dtype)))
        n_params, n_outs = len(in_names), len(out_avals)
        all_names = in_names + out_names + ([pname] if pname else [])
        donate = tuple(range(n_params, n_params + n_outs))

        def _body(*args):
            operands = list(args)
            if pname is not None:
                operands.append(_b2j.partition_id_tensor())
            return tuple(_b2j._bass_exec_p.bind(
                *operands, out_avals=tuple(out_avals),
                in_names=tuple(all_names), out_names=tuple(out_names),
                lowering_input_output_aliases=(),
                sim_require_finite=True, sim_require_nnan=True, nc=nc))

        devices = jax.devices()[:n_cores]
        assert len(devices) == n_cores
        mesh = _b2j.Mesh(np.asarray(devices), ("core",))
        sharded = jax.jit(
            _b2j.shard_map(
                _body, mesh=mesh,
                in_specs=(_b2j.PartitionSpec("core"),) * (n_params + n_outs),
                out_specs=(_b2j.PartitionSpec("core"),) * n_outs,
                check_rep=False),
            donate_argnums=donate, keep_unused=True)
        ent = (sharded, in_names, out_names, out_avals)
        _RUNNER_CACHE[id(nc)] = ent
    sharded, in_names, out_names, out_avals = ent
    ckey = tuple(id(in_maps[c][nm]) for nm in in_names
                 for c in range(n_cores))
    cent = _CONCAT_CACHE.get(id(nc))
    if cent is None or cent[0] != ckey:
        cent = (ckey, [
            np.concatenate(
                [np.asarray(in_maps[c][nm]) for c in range(n_cores)],
                axis=0) for nm in in_names])
        _CONCAT_CACHE[id(nc)] = cent
    concat_in = cent[1]
    concat_zeros = [
        np.zeros((n_cores * av.shape[0], *av.shape[1:]), av.dtype)
        for av in out_avals]
    out_arrs = sharded(*concat_in, *concat_zeros)
    hook = _STREAM_HOOK[0]
    if hook is not None and len(out_arrs) == 1:
        # stream per-core shards: downloads serialize on the client, so
        # run the consumer on a worker thread to overlap postprocessing
        # with the remaining transfers.
        from concurrent.futures import ThreadPoolExecutor
        rows = out_avals[0].shape[0]
        results = [dict() for _ in range(n_cores)]
        out_arrs[0].copy_to_host_async()   # pipeline all shard copies
        with ThreadPoolExecutor(1) as ex:
            futs = []
            for shard in out_arrs[0].addressable_shards:
                c = shard.index[0].start // rows
                data = np.asarray(shard.data)
                results[c][out_names[0]] = data
                futs.append(ex.submit(hook, c, data))
            for f in futs:
                f.result()
        return results
    np_outs = jax.device_get(list(out_arrs))     # one fetch per output
    return [
        {nm: np_outs[i].reshape(n_cores, *out_avals[i].shape)[c]
         for i, nm in enumerate(out_names)}
        for c in range(n_cores)
    ]


_b2j.run_bass_via_pjrt = _patched_run_bass_via_pjrt
# -------------------------------------------------------------------------


class Cfg:
    def __init__(self, n_nodes, n_feat, n_cores, kk, gmax=8):
        assert n_nodes % n_cores == 0
        self.N = n_nodes
        self.F = n_feat
        self.NC = n_cores
        self.NS = n_nodes // n_cores          # dests per core
        self.NBLK = -(-self.NS // P)          # blocks per core
        self.NPB = n_cores * self.NBLK * P    # permuted z2 table rows
        # kk = (K0a, K1a, K0b, K1b): win chunks per block, passes 1 and 2
        self.KK = kk
        self.GMAX = gmax                      # chunks per dma_gather
        self.SGRP = 8                         # blocks per staging group
        self.NSG = -(-self.NBLK // self.SGRP)
        self.NG = tuple(-(-(self.NBLK * k) // gmax) for k in kk)
        self.WIN = 32768 if n_nodes > 32768 else max(P, n_nodes // 2)


def _best_blocks(r_loc, win_e, ns, nblk):
    """Assign dests 0..ns-1 to nblk blocks of <=P slots.  Serpentine
    round-robin over degree-sorted dests, tried with three sort keys
    (total / win1 / win0 degree); picks the one minimizing the padded
    chunk capacity k0+k1 of the windowed layout.
    Returns (block_of[ns], pos_of[ns])."""
    deg_t = np.bincount(r_loc, minlength=ns)
    deg_1 = np.bincount(r_loc[win_e == 1], minlength=ns)
    i = np.arange(ns)
    r = (i // nblk).astype(np.int32)
    c = i % nblk
    blk = np.where(r % 2 == 0, c, nblk - 1 - c).astype(np.int32)
    best = None
    for key in (deg_t, deg_1, deg_t - deg_1):
        order = np.argsort(-key, kind="stable")
        bo = np.empty(ns, np.int32)
        po = np.empty(ns, np.int32)
        bo[order] = blk
        po[order] = r
        b_e = bo[r_loc]
        c0 = np.bincount(b_e[win_e == 0], minlength=nblk).max(initial=1)
        c1 = np.bincount(b_e[win_e == 1], minlength=nblk).max(initial=1)
        k0 = max(1, -(-int(c0) // P))
        k1 = max(1, -(-int(c1) // P))
        score = (k0 + k1, k0)
        if best is None or score < best[0]:
            best = (score, bo, po)
    return best[1], best[2]


def _pass_layout(b_e, dl_e, cp, w, win_thr, nblk):
    """Sort edges by (block, window); count per-(block,window); return the
    sorted streams and needed chunk capacities."""
    win_e = (cp >= win_thr).astype(np.int32)
    order = np.argsort(b_e.astype(np.int32) * 2 + win_e, kind="stable")
    b_s, win_s, dl_s, c_s, w_s = (
        b_e[order], win_e[order], dl_e[order], cp[order], w[order])
    cnt = np.bincount(b_s * 2 + win_s, minlength=nblk * 2).reshape(nblk, 2)
    k0 = max(1, int(np.ceil(cnt[:, 0].max() / P))) if len(b_e) else 1
    k1 = max(1, int(np.ceil(cnt[:, 1].max() / P))) if len(b_e) else 1
    return dict(b=b_s, win=win_s, dl=dl_s, c=c_s, w=w_s, cnt=cnt,
                k0=k0, k1=k1)


def _encode_pass(pl, K0, K1, NG0, NG1, cfg):
    """idx0/idx1 [16, NG*GM*8] compact gather id streams + meta fp16."""
    nblk, K, GM = cfg.NBLK, K0 + K1, cfg.GMAX
    idx0 = np.zeros((nblk, K0 * P), np.int16)
    idx1 = np.zeros((nblk, K1 * P), np.int16)
    meta = np.zeros((P, nblk, K, 2), np.float16)
    cnt = pl["cnt"]
    starts = np.zeros(nblk * 2, np.int64)
    starts[1:] = np.cumsum(cnt.reshape(-1))[:-1]
    key = pl["b"] * 2 + pl["win"]
    iw = np.arange(len(key)) - starts[key]
    b, win, dl, c, w = pl["b"], pl["win"], pl["dl"], pl["c"], pl["w"]
    m0 = win == 0
    idx0[b[m0], iw[m0]] = c[m0].astype(np.int16)
    m1 = ~m0
    idx1[b[m1], iw[m1]] = (c[m1] - cfg.WIN).astype(np.int16)
    kk = np.where(m0, iw // P, K0 + iw // P)
    meta[iw % P, b, kk, 0] = dl
    meta[iw % P, b, kk, 1] = w

    def enc(idx, Kw, n_gath):
        stream = np.zeros(n_gath * GM * P, np.int16)
        s = idx.reshape(-1)
        stream[:s.size] = s
        return np.ascontiguousarray(stream.reshape(n_gath * GM * 8, 16).T)

    return (enc(idx0, K0, NG0), enc(idx1, K1, NG1),
            np.ascontiguousarray(meta.reshape(P, nblk * K * 2)))


def _build_program(cfg, phases="ABCDE"):
    N, F, NC = cfg.N, cfg.F, cfg.NC
    NS, NBLK = cfg.NS, cfg.NBLK
    NPB = cfg.NPB
    K0a, K1a, K0b, K1b = cfg.KK
    Ka, Kb = K0a + K1a, K0b + K1b
    NGa0, NGa1, NGb0, NGb1 = cfg.NG
    NW0 = min(N, cfg.WIN)
    f32 = mybir.dt.float32
    f16 = mybir.dt.float16
    i8 = mybir.dt.int8
    GM = cfg.GMAX
    SG = cfg.SGRP

    nc = bacc.Bacc("TRN2", target_bir_lowering=False, debug=False,
                   enable_asserts=False, num_devices=NC, num_swdge_queues=4)

    # ---- inputs ----------------------------------------------------------
    xsT = nc.dram_tensor("xsT", [F, NBLK * P], f16, kind="ExternalInput").ap()
    WT = nc.dram_tensor("WT", [3 * F, F], f16, kind="ExternalInput").ap()
    B16 = nc.dram_tensor("B16", [3, F], f16, kind="ExternalInput").ap()
    ins = {}
    for nm, ng0, ng1, kk in (("a", NGa0, NGa1, Ka), ("b", NGb0, NGb1, Kb)):
        ins[f"idx0{nm}"] = nc.dram_tensor(
            f"idx0{nm}", [16, ng0 * GM * 8], mybir.dt.int16,
            kind="ExternalInput").ap()
        ins[f"idx1{nm}"] = nc.dram_tensor(
            f"idx1{nm}", [16, ng1 * GM * 8], mybir.dt.int16,
            kind="ExternalInput").ap()
        ins[f"meta{nm}"] = nc.dram_tensor(
            f"meta{nm}", [P, NBLK * kk * 2], f16, kind="ExternalInput").ap()

    # ---- outputs / scratch ----------------------------------------------
    # rows 0:NBLK*P = int8 payload; last P rows carry the f16 scale table
    # [P, 3*NBLK] bitcast into bytes (cols 0:6*NBLK)
    q_buf = nc.dram_tensor("q", [NBLK * P + P, 3 * F], i8,
                           kind="ExternalOutput").ap()
    zsh = nc.dram_tensor("zsh", [NS, 2 * F], f16, kind="Internal").ap()
    table = nc.dram_tensor("table", [N, 2 * F], f16, kind="Internal",
                           addr_space="Shared").ap()
    z2b = nc.dram_tensor("z2b", [NBLK * P, F], f16, kind="Internal").ap()
    z2t = nc.dram_tensor("z2t", [NPB, F], f16, kind="Internal",
                         addr_space="Shared").ap()

    qctr = [0]

    def next_queue():
        q = qctr[0] % 4
        qctr[0] += 1
        return q

    with tile.TileContext(nc) as tc:
        with tc.tile_pool(name="const", bufs=1) as cpool, \
             tc.tile_pool(name="rs", bufs=4) as rpool:
            iota_i = cpool.tile([P, P], mybir.dt.int16)
            nc.gpsimd.iota(iota_i[:], pattern=[[1, P]], base=0,
                           channel_multiplier=0)
            iota_t = cpool.tile([P, P], f16)
            nc.vector.tensor_copy(iota_t[:], iota_i[:])
            xs_t = cpool.tile([F, NBLK * P], f16)
            nc.sync.dma_start(xs_t[:], xsT[:])
            sca_t = cpool.tile([P, 3 * NBLK], f16)
            nc.vector.memset(sca_t[:], 0.0)
            tiles = {}
            for nm, ng0, ng1, kk in (("a", NGa0, NGa1, Ka),
                                     ("b", NGb0, NGb1, Kb)):
                m16 = cpool.tile([P, NBLK * kk * 2], f16, tag=f"m16{nm}",
                                 name=f"m16{nm}")
                nc.sync.dma_start(m16[:], ins[f"meta{nm}"][:])
                mt = cpool.tile([P, NBLK * kk * 2], f32, tag=f"mt{nm}",
                                name=f"mt{nm}")
                nc.vector.tensor_copy(mt[:], m16[:])
                tiles[f"meta{nm}"] = mt
                for w_, ng in (("0", ng0), ("1", ng1)):
                    ix = cpool.tile([P, ng * GM * 8], mybir.dt.int16,
                                    tag=f"ix{w_}{nm}", name=f"ix{w_}{nm}")
                    for g in range(8):
                        nc.sync.dma_start(ix[16 * g:16 * (g + 1), :],
                                          ins[f"idx{w_}{nm}"][:])
                    tiles[f"ix{w_}{nm}"] = ix
            wt_t = []
            b16_t = []
            for j in range(3):
                wtj = cpool.tile([F, F], f16, tag=f"wt{j}", name=f"wt{j}")
                b16j = cpool.tile([1, F], f16, tag=f"b16{j}", name=f"b16{j}")
                nc.sync.dma_start(wtj[:], WT[j * F:(j + 1) * F, :])
                nc.sync.dma_start(b16j[:], B16[j:j + 1, :])
                wt_t.append(wtj)
                b16_t.append(b16j)
            ones_t = cpool.tile([1, P], f16)
            nc.vector.memset(ones_t[:], 1.0)

            def quantize(ps_slice, w_, scol, qout):
                """abs-rowmax -> sca_t[:, scol], qout = int8(ps*QMAX/rmax)."""
                nc.vector.tensor_reduce(
                    out=sca_t[:w_, scol:scol + 1], in_=ps_slice,
                    axis=mybir.AxisListType.X, op=mybir.AluOpType.max,
                    apply_absolute_value=True)
                rs = rpool.tile([P, 1], f32, tag="rs")
                nc.vector.tensor_scalar(
                    out=rs[:w_], in0=sca_t[:w_, scol:scol + 1],
                    scalar1=1e-20, scalar2=None, op0=mybir.AluOpType.max)
                nc.vector.reciprocal(rs[:w_], rs[:w_])
                nc.vector.tensor_scalar(
                    out=qout, in0=ps_slice, scalar1=rs[:w_], scalar2=QMAX,
                    op0=mybir.AluOpType.mult, op1=mybir.AluOpType.mult)

            # ---- Phase AB: project own shard with W0|W1|W2 ---------------
            # y0 (int8 + scales) to qy_buf; [z1|z2] (fp16) to zsh.
            if "A" in phases:
             with tc.tile_pool(name="projAB", bufs=3) as apool, \
                  tc.tile_pool(name="psumAB", bufs=3, space="PSUM") as apsum:
                for t in range(NBLK):
                    r0 = t * P
                    r1 = min(NS, r0 + P)
                    w_ = r1 - r0
                    if w_ <= 0:
                        break
                    ps = apsum.tile([P, 3 * F], f32, space="PSUM")
                    for j in range(3):
                        nc.tensor.matmul(
                            ps[:w_, j * F:(j + 1) * F],
                            lhsT=xs_t[:, r0:r0 + w_], rhs=wt_t[j][:],
                            start=True, stop=False)
                        nc.tensor.matmul(
                            ps[:w_, j * F:(j + 1) * F],
                            lhsT=ones_t[:, :w_], rhs=b16_t[j][:],
                            start=False, stop=True)
                    qt = apool.tile([P, F], i8, tag="qt")
                    quantize(ps[:w_, 0:F], w_, t, qt[:w_, :])
                    nc.sync.dma_start(q_buf[r0:r1, 0:F], qt[:w_, :])
                    st = apool.tile([P, 2 * F], f16, tag="stab")
                    if t % 2 == 0:
                        nc.vector.tensor_copy(st[:w_, :], ps[:w_, F:3 * F])
                    else:
                        nc.scalar.copy(st[:w_, :], ps[:w_, F:3 * F])
                    nc.sync.dma_start(zsh[r0:r1, :], st[:w_, :])

            # ---- Phase B: AllGather table shards -------------------------
            if "B" in phases:
                nc.gpsimd.collective_compute(
                    "AllGather", mybir.AluOpType.bypass,
                    replica_groups=[list(range(NC))],
                    ins=[zsh[:]], outs=[table[:]],
                )

            # ---- SpMM machinery ------------------------------------------
            def spmm(src_w0, src_w1, fdim, dsts, scol0, ix0_t, ix1_t,
                     meta_t, K0, K1):
                """Gathers stream GM-chunk slices of the block-major chunk
                stream per window; segment matmuls accumulate per block in
                PSUM; per-block int8 quantization (+ fp16 copy for z2);
                block-order [P, SG*F] DMA writes (host/pass-2 indices undo
                the block permutation -- no scatter)."""
                K = K0 + K1
                with tc.tile_pool(name="ga", bufs=6) as gapool, \
                     tc.tile_pool(name="sS", bufs=32) as spool, \
                     tc.tile_pool(name="stg", bufs=3) as stgpool, \
                     tc.tile_pool(name="psC", bufs=4, space="PSUM") as cpsum:
                    wins = [[src_w0, ix0_t, NBLK * K0, [], 0],
                            [src_w1, ix1_t, NBLK * K1, [], 0]]

                    def ensure_gathers(w, upto_chunk):
                        src_w, ix_t, tot, gtiles, _ = wins[w]
                        while wins[w][4] * GM < min(upto_chunk, tot):
                            g = wins[w][4]
                            cg = min(GM, tot - GM * g)
                            ga = gapool.tile([P, GM, fdim], f16,
                                             tag=f"ga{w}", name=f"ga{w}_{g}")
                            nc.gpsimd.dma_gather(
                                ga[:, :cg, :], src_w,
                                ix_t[:, g * GM * 8: g * GM * 8 + cg * 8],
                                num_idxs=cg * P, num_idxs_reg=cg * P,
                                elem_size=fdim, queue_num=next_queue())
                            gtiles.append(ga)
                            wins[w][4] += 1

                    stgs = None
                    for b in range(NBLK):
                        g_s, c_s = b // SG, b % SG
                        nb = min(SG, NBLK - g_s * SG)
                        if c_s == 0:
                            stgs = [stgpool.tile(
                                        [P, SG, F],
                                        i8 if kind == "quant" else f16,
                                        tag=f"stg{i}", name=f"stg{i}_{g_s}")
                                    for i, (kind, _) in enumerate(dsts)]
                        ensure_gathers(0, (b + 1) * K0)
                        ensure_gathers(1, (b + 1) * K1)
                        ps = cpsum.tile([P, fdim], f32, space="PSUM")
                        for k in range(K):
                            S = spool.tile([P, P], f16, tag="S")
                            mo = (b * K + k) * 2
                            nc.vector.tensor_scalar(
                                out=S[:], in0=iota_t[:],
                                scalar1=meta_t[:, mo:mo + 1],
                                scalar2=meta_t[:, mo + 1:mo + 2],
                                op0=mybir.AluOpType.is_equal,
                                op1=mybir.AluOpType.mult)
                            if k < K0:
                                gk = b * K0 + k
                                rhs = wins[0][3][gk // GM][:, gk % GM, :]
                            else:
                                gk = b * K1 + (k - K0)
                                rhs = wins[1][3][gk // GM][:, gk % GM, :]
                            nc.tensor.matmul(ps[:], lhsT=S[:], rhs=rhs,
                                             start=(k == 0),
                                             stop=(k == K - 1))
                        for i, (kind, dst) in enumerate(dsts):
                            if kind == "quant":
                                quantize(ps[:, i * F:(i + 1) * F], P,
                                         scol0 + b, stgs[i][:, c_s, :])
                            else:
                                nc.scalar.copy(stgs[i][:, c_s, :],
                                               ps[:, i * F:(i + 1) * F])
                        if c_s == nb - 1:
                            r0 = g_s * SG * P
                            r1 = r0 + nb * P
                            for i, (kind, dst) in enumerate(dsts):
                                nc.sync.dma_start(
                                    dst[r0:r1, :].rearrange(
                                        "(b p) f -> p b f", p=P),
                                    stgs[i][:, :nb, :])

            # ---- Phase C: SpMM1 over table -> q[:, F:2F], z2b ------------
            if "C" in phases:
                spmm(table[:NW0, :], table[cfg.WIN:N, :], 2 * F,
                     [("quant", q_buf[:, F:2 * F]), ("f16", z2b[:])], NBLK,
                     tiles["ix0a"], tiles["ix1a"], tiles["metaa"], K0a, K1a)

            # ---- Phase D: AllGather z2 block buffers ---------------------
            if "D" in phases:
                nc.gpsimd.collective_compute(
                    "AllGather", mybir.AluOpType.bypass,
                    replica_groups=[list(range(NC))],
                    ins=[z2b[:]], outs=[z2t[:]],
                )

            # ---- Phase E: SpMM2 over permuted z2 -> q[:, 2F:3F] ----------
            if "E" in phases:
                spmm(z2t[:NW0, :], z2t[cfg.WIN:NPB, :], F,
                     [("quant", q_buf[:, 2 * F:3 * F])], 2 * NBLK,
                     tiles["ix0b"], tiles["ix1b"], tiles["metab"], K0b, K1b)

            # ---- scales out (bitcast f16 -> bytes, last P rows of q) -----
            nc.sync.dma_start(
                q_buf[NBLK * P:NBLK * P + P, 0:6 * NBLK],
                sca_t[:].bitcast(i8))

    nc.compile()
    return nc


_CACHE = {}


def _get_program(cfg, phases="ABCDE"):
    key = (cfg.N, cfg.F, cfg.NC, cfg.KK, cfg.GMAX, phases)
    if key not in _CACHE:
        _CACHE[key] = _build_program(cfg, phases)
    return _CACHE[key]


def _prepare(x, edge_weight, W, b, row, col, n_cores=8):
    N, F = np.asarray(x).shape
    row = np.asarray(row).astype(np.int64)
    col = np.asarray(col).astype(np.int64)
    w = np.asarray(edge_weight).astype(np.float32)
    x = np.asarray(x).astype(np.float32)
    W = np.asarray(W).astype(np.float32)
    b = np.asarray(b).astype(np.float32)

    ns = N // n_cores
    nblk = -(-ns // P)
    core_of = row // ns

    # block assignment per core + permuted z2 id map
    win_thr = 32768 if N > 32768 else max(P, N // 2)
    edges = []
    bos, pos = [], []
    pmap = np.empty(N, np.int32)
    for m in range(n_cores):
        sel = np.where(core_of == m)[0]
        r_loc = (row[sel] - m * ns).astype(np.int64)
        c_e = col[sel]
        win_a = (c_e >= win_thr).astype(np.int32)
        bo, po = _best_blocks(r_loc, win_a, ns, nblk)
        bos.append(bo)
        pos.append(po)
        pmap[m * ns:(m + 1) * ns] = m * nblk * P + bo * P + po
        edges.append((bo[r_loc], po[r_loc], c_e, w[sel]))
    pls_a, pls_b = [], []
    for m in range(n_cores):
        b_e, dl_e, c_e, w_e = edges[m]
        pls_a.append(_pass_layout(b_e, dl_e, c_e, w_e, win_thr, nblk))
        pls_b.append(_pass_layout(b_e, dl_e, pmap[c_e], w_e, win_thr, nblk))
    kk = (max(pl["k0"] for pl in pls_a), max(pl["k1"] for pl in pls_a),
          max(pl["k0"] for pl in pls_b), max(pl["k1"] for pl in pls_b))
    cfg = Cfg(N, F, n_cores, kk)

    xT16 = x.T.astype(np.float16)                          # [F, N]
    WT = np.ascontiguousarray(
        np.transpose(W, (0, 2, 1))).reshape(3 * F, F).astype(np.float16)
    B16 = np.ascontiguousarray(b.astype(np.float16))       # [3, F]

    NGa0, NGa1, NGb0, NGb1 = cfg.NG
    in_maps = []
    for m in range(n_cores):
        i0a, i1a, ma = _encode_pass(pls_a[m], kk[0], kk[1], NGa0, NGa1, cfg)
        i0b, i1b, mb = _encode_pass(pls_b[m], kk[2], kk[3], NGb0, NGb1, cfg)
        xs = np.zeros((F, cfg.NBLK * P), np.float16)
        xs[:, :ns] = xT16[:, m * ns:(m + 1) * ns]
        in_maps.append(dict(
            xsT=xs, WT=WT, B16=B16,
            idx0a=i0a, idx1a=i1a, metaa=ma,
            idx0b=i0b, idx1b=i1b, metab=mb,
        ))
    return cfg, in_maps, (bos, pos)


def kernel(x, edge_weight, W, b, row, col):
    n_cores = 8
    N, F = np.asarray(x).shape
    ns = N // n_cores
    cfg, in_maps, (bos, pos) = _prepare(x, edge_weight, W, b, row, col,
                                        n_cores)
    nc = _get_program(cfg)
    NBLK = cfg.NBLK
    NR = NBLK * P
    out = np.empty((N, 3 * F), np.float32)

    def dequant(m, q):
        sca = (np.ascontiguousarray(q[NR:NR + P, 0:6 * NBLK])
               .view(np.float16).astype(np.float32) / QMAX)  # [P, 3*NBLK]
        dst = out[m * ns:(m + 1) * ns]
        # y0 rows are tile-sequential: row r -> (slot r%P, tile r//P)
        s0 = sca[:, 0:NBLK].T.reshape(-1)[:ns]
        np.multiply(q[:ns, 0:F], s0[:, None], out=dst[:, 0:F])
        bo, po = bos[m], pos[m]
        slot = bo * P + po                           # block-permuted row
        np.multiply(q[slot, F:2 * F], sca[po, NBLK + bo][:, None],
                    out=dst[:, F:2 * F])
        np.multiply(q[slot, 2 * F:3 * F], sca[po, 2 * NBLK + bo][:, None],
                    out=dst[:, 2 * F:3 * F])

    _STREAM_HOOK[0] = dequant
    try:
        bass_utils.run_bass_kernel_spmd(nc, in_maps,
                                        core_ids=list(range(n_cores)))
    finally:
        _STREAM_HOOK[0] = None
    return out
